# revision 46
# baseline (speedup 1.0000x reference)
"""BertSelfAttention (relative_key_query) Trainium2 Bass kernel.

Sharding: 8 cores = 4 batches x 2 head-groups (8 heads each); each core is
fully independent (no collectives).

Score layout is TRANSPOSED on-chip: scoresT[r, l] (r on partitions), so
probs @ V needs no transpose of probs, and the softmax denominator falls
out of an appended ones-column in the PV matmul.

Relative-position bias ("relative_key_query"):
  bias1[l,r] = q[l] . dist_emb[l-r+1023]
  bias2[l,r] = k[r] . dist_emb[l-r+1023]
computed as banded K=64 row-packed matmuls against the distance tables and
realigned via a SHEARED fp8 DRAM scratch (a DRAM-side shear is the only
mechanism on TRN2 that can express the (l-r) diagonal gather).  The TWO
HEADS of a pair are packed as the two BYTES of one bf16 container:
  qb12 row l of pair p: f8 col 2*(128+r)+h%2 = bias1_h[l,r]
  kb12 row r of pair p: f8 col 2*(128+l)+h%2 = bias2_h[l,r]
so each pair reads back with ONE bf16 xbar DMA-transpose (qd) plus ONE
plain bf16 row read (kd) -> [r-part, rt, l] tiles; fp8 halves the scratch
traffic vs a bf16 layout.  The bias add happens ON THE PE as two identity
matmuls per score chunk with a stride-2 fp8 rhs selecting the byte lane
(no GPSIMD pre-add, no unpack pass).

exp(scores/8) runs on ScalarE straight out of PSUM into bf16 probs; pv runs
bf16 with the ones-column producing the denominator.  The context ships to
DRAM UNNORMALIZED and TRANSPOSED ([head, d(65), l], row 64 = denominator):
the divide and the [l, h*64+d] unshuffle happen on the host, which removes
the whole PE-transpose + reciprocal + scale output stage from the device.

Scheduling (the HAM clock gate at K=4/8 half-rate was the dominant v1
loss): phase A (projections) weaves the band matmuls for pairs 0/1 between
projection matmuls with double-buffered PSUM so evacuation latency never
enters the PE queue; its scoped pools end in a barrier that drains the
DVE/ACT evacuation backlog before the head phase starts.  Heads 0-3 carry
pairs 2/3's band steps as weave filler; per-pair bias reads prefetch two
heads ahead; pv is deferred a few steps so exp is off the PE critical
path; v-projection evacs alternate DVE/ACT so neither backlog gates head
0.  DMA rings: sync = weights + qd shears + transposes + output, scalar =
activations + kb12, gpsimd(SWDGE) = tables + kd shears.
"""

import numpy as np

B, S, H = 4, 1024, 1024
NH, HS = 16, 64
NHL = 8            # heads per core
NPAIR = 4          # head-pairs per core
BAND = 1152        # banded width of qd'/kd per 128-row tile (1151 used + 1 pad)
RSF = 2560         # scratch row pitch in f8 bytes (= 2*BAND + 256 exactly)
TSPF = 128 * RSF   # f8 elements per 128-row block
HSPF = S * RSF     # f8 elements per head-pair
NCORES = 8

_CACHE = {}


def _build_program():
    import concourse.bass as bass
    import concourse.mybir as mybir
    import concourse.tile as tile
    from concourse import bacc

    f32 = mybir.dt.float32
    bf16 = mybir.dt.bfloat16
    f8 = mybir.dt.float8e4
    AF = mybir.ActivationFunctionType
    ALU = mybir.AluOpType

    nc = bacc.Bacc("TRN2", debug=False)

    hsT = nc.dram_tensor("hsT", [H, S], bf16, kind="ExternalInput").ap()
    wT = nc.dram_tensor("wT", [H, 3 * 512], bf16, kind="ExternalInput").ap()
    det = nc.dram_tensor("det", [HS, 2048], bf16, kind="ExternalInput").ap()
    rdt = nc.dram_tensor("rdt", [HS, 2048], bf16, kind="ExternalInput").ap()
    # transposed UNNORMALIZED output [head, d, l]; row 64 is the softmax
    # denominator; the host does the divide and the [l, h*64+d] unshuffle
    outT = nc.dram_tensor("outT", [NHL, HS + 1, S], f32, kind="ExternalOutput").ap()
    # f8 scratch; row l of pair p holds bias1 for heads (2p, 2p+1) byte-packed
    qb12 = nc.dram_tensor("qb12", [NPAIR, S, RSF], f8)
    kb12 = nc.dram_tensor("kb12", [NPAIR, S, RSF], f8)

    with tile.TileContext(nc) as tc:
        # single top-level pool scope: no mid-kernel pool closes, hence no
        # cross-engine barriers between the projection and score phases
        with tc.tile_pool(name="const", bufs=1) as constp, \
             tc.tile_pool(name="qkv", bufs=1) as qkvp, \
             tc.tile_pool(name="bandp", bufs=2) as bandp, \
             tc.tile_pool(name="psB", bufs=2, space="PSUM") as psB, \
             tc.tile_pool(name="psS", bufs=2, space="PSUM") as psS, \
             tc.tile_pool(name="psC", bufs=1, space="PSUM") as psC:
            # dist tables duplicated on partitions [0:64] and [64:128] so the
            # K=64 head-pair matmuls can row-pack (lhsT/rhs same base partition)
            det_sb = constp.tile([128, 2048], bf16)
            rdt_sb = constp.tile([128, 2048], bf16)
            i128 = constp.tile([128, 128], f8)       # fp8 identity
            onesb = constp.tile([128, 1], bf16)
            nc.gpsimd.dma_start(out=det_sb[0:64, :], in_=det[:])
            nc.gpsimd.dma_start(out=det_sb[64:128, :], in_=det[:])
            nc.gpsimd.dma_start(out=rdt_sb[0:64, :], in_=rdt[:])
            nc.gpsimd.dma_start(out=rdt_sb[64:128, :], in_=rdt[:])
            nc.gpsimd.memset(i128[:], 0.0)
            nc.gpsimd.affine_select(
                out=i128[:], in_=i128[:],
                compare_op=mybir.AluOpType.not_equal,
                fill=1.0, base=0,
                pattern=[[-1, 128]], channel_multiplier=1)
            nc.vector.memset(onesb[:], 1.0)

            # persistent per-core activations (layout [part=(h%2)*64+d, h//2, l])
            qT8 = qkvp.tile([128, 4, S], bf16)
            kT8 = qkvp.tile([128, 4, S], bf16)
            # bf16 v; col 64 = ones (softmax denominator via pv matmul)
            v_sb = qkvp.tile([128, 8, NHL, 66], bf16)
            nc.gpsimd.memset(v_sb[:], 1.0)

            CHUNKS = ((0, 512), (512, 512), (1024, 128))

            def make_band_steps(hp, which, t):
                """5 composite step-closures: per chunk, both subs' MMs into
                one single-bank psB tile + ONE byte-interleaving evac; the
                last also issues the sheared f8 DRAM write.  Evac engine
                split is phase-aware: phase A (pairs 0,1) gives ACT 3/5
                chunks; the head phase (pairs 2,3) gives ACT only 1/5 so
                exp() never queues behind band evacuation."""
                src8, tab, dst = (
                    (qT8, rdt_sb, qb12) if which == 0
                    else (kT8, det_sb, kb12))
                c0 = 896 - 128 * t
                band = bandp.tile([128, 2 * BAND], f8,
                                  tag=f"band{which}",
                                  name=f"band_{hp}_{which}_{t}")
                # view [part, sub-byte, c]: element (sub, c) -> f8 col 2c+sub
                band_i = band[:].rearrange("p (c two) -> p two c", two=2)
                act_chunks = ((t + which) % 2,)
                steps = []
                for ci, (coff, w) in enumerate(CHUNKS):
                    def step(ci=ci, coff=coff, w=w, last=(ci == 2)):
                        p = psB.tile([128, 2, 512], f32, tag="pqd")
                        for sub in range(2):
                            bp = 64 * sub
                            nc.tensor.matmul(
                                p[:, sub, 0:w],
                                src8[bp:bp + 64, hp, t * 128:(t + 1) * 128],
                                tab[bp:bp + 64, c0 + coff: c0 + coff + w],
                                start=True, stop=True)
                        dst_sl = band_i[:, :, coff:coff + w]
                        if ci in act_chunks:
                            nc.scalar.copy(dst_sl, p[:, :, 0:w])
                        else:
                            nc.vector.tensor_copy(dst_sl, p[:, :, 0:w])
                        if last:
                            shear = bass.AP(
                                tensor=dst,
                                offset=hp * HSPF + t * TSPF + 2,
                                ap=[[RSF + 2, 128], [1, 2 * BAND]])
                            if which == 0 and hp < 2:
                                nc.sync.dma_start(out=shear, in_=band[:])
                            else:
                                # SWDGE on the otherwise-idle GPSIMD queue:
                                # keeps the ACT engine free for exp/evac
                                nc.gpsimd.dma_start(out=shear, in_=band[:])
                    steps.append(step)
                return steps

            # ---------- Phase A: QKV projections ----------
            hsT_sb = qkvp.tile([128, 8, S], bf16)
            wT_sb = qkvp.tile([128, 8, 3 * 512], bf16)
            # warm-up burst: tiny matmuls keep the PE HAM counter busy
            # while the input DMAs land, so projections start at 2.4 GHz
            for wi in range(128):
                wps = psS.tile([1, 4], f32, tag="ps", name=f"warm_{wi}")
                nc.tensor.matmul(wps[:, 0:1], onesb[:, 0:1], onesb[:],
                                 start=True, stop=True)
            phase_steps = []

            def pweave():
                if phase_steps:
                    phase_steps.pop(0)()

            hsT_r = hsT.rearrange("(a p) l -> p a l", p=128)
            wT_r = wT.rearrange("(a p) n -> p a n", p=128)
            for j in range(8):
                nc.sync.dma_start(out=wT_sb[:, j, :], in_=wT_r[:, j, :])
                nc.scalar.dma_start(out=hsT_sb[:, j, :], in_=hsT_r[:, j, :])

            def emit_proj(sel, dst8):
                # qT / kT: out[o, l] = sum_j W[o, j] hs[l, j].  ALL FOUR
                # head-pairs' band steps are emitted here: 192 total steps
                # exactly fill the 192 projection weave points, so the
                # entire bias pipeline runs inside phase A and the head
                # phase is a clean PE-paced stream.
                for ot in range(4):
                    ps2 = [psS.tile([128, 512], f32, tag="ps",
                                    name=f"pa_{sel}_{ot}_{lc}")
                           for lc in range(2)]
                    for j in range(8):
                        for lc in range(2):
                            nc.tensor.matmul(
                                ps2[lc][:],
                                wT_sb[:, j, sel * 512 + ot * 128: sel * 512 + (ot + 1) * 128],
                                hsT_sb[:, j, lc * 512:(lc + 1) * 512],
                                start=(j == 0), stop=(j == 7))
                            pweave()
                    for lc in range(2):
                        nc.vector.tensor_copy(dst8[:, ot, lc * 512:(lc + 1) * 512], ps2[lc][:])
                    if ot < 2:
                        phase_steps.extend(
                            st for t in range(8)
                            for st in make_band_steps(ot, sel, t))

            emit_proj(0, qT8)
            emit_proj(1, kT8)
            for rt in range(8):
                p = psS.tile([128, 512], f32, tag="ps", name=f"pav_{rt}")
                for j in range(8):
                    nc.tensor.matmul(
                        p[:],
                        hsT_sb[:, j, rt * 128:(rt + 1) * 128],
                        wT_sb[:, j, 1024:1536],
                        start=(j == 0), stop=(j == 7))
                    pweave()
                nc.vector.tensor_copy(
                    v_sb[:, rt, :, 0:64],
                    p[:].rearrange("p (h d) -> p h d", h=NHL))
            while phase_steps:
                phase_steps.pop(0)()

            # ---------- Phases B+C interleaved ----------
            with tc.tile_pool(name="b1p", bufs=3) as b1p, \
                 tc.tile_pool(name="b2p", bufs=3) as b2p, \
                 tc.tile_pool(name="exp", bufs=9) as exp_p, \
                 tc.tile_pool(name="ctxp", bufs=2) as ctxp, \
                 tc.tile_pool(name="outp", bufs=2) as outp:

                btiles = {}

                def emit_trans(pr):
                    # bias1 for pair pr: ONE bf16 xbar transpose-read.
                    # partition p of group g holds heads (2pr, 2pr+1) byte-
                    # packed bias1[l, r=128g+p] at bf16 col l.
                    t1 = b1p.tile([128, 8, S], bf16, tag="b1", name=f"b1_{pr}")
                    src = bass.AP(
                        tensor=qb12,
                        offset=pr * HSPF + 256,
                        ap=[[RSF, S], [1, 2048]]).bitcast(bf16)
                    nc.sync.dma_start_transpose(t1[:], src)
                    btiles[(0, pr)] = t1

                def emit_read(pr):
                    # bias2 for pair pr: ONE plain bf16 read of sheared rows
                    # (sync queue: keeps the ACT engine exp-only)
                    t2 = b2p.tile([128, 8, S], bf16, tag="b2", name=f"b2_{pr}")
                    src = bass.AP(
                        tensor=kb12,
                        offset=pr * HSPF + 256,
                        ap=[[RSF, 128], [TSPF, 8], [1, 2048]]).bitcast(bf16)
                    nc.scalar.dma_start(out=t2[:], in_=src)
                    btiles[(1, pr)] = t2

                def emit_head(h, steps):
                    hp, sub = h // 2, h % 2
                    bp = 64 * sub
                    b1t = btiles[(0, hp)]
                    b2t = btiles[(1, hp)]
                    # f8 views [part, g, l, byte]; byte lane = head parity
                    b1f = b1t[:].bitcast(f8).rearrange(
                        "p g (l two) -> p g l two", two=2)
                    b2f = b2t[:].bitcast(f8).rearrange(
                        "p g (l two) -> p g l two", two=2)
                    pc_ = psC.tile([65, 512], f32, tag="pc", name=f"pc_{h}")
                    pending = []
                    exs1 = []
                    allsteps = list(steps)

                    def weave(k=1):
                        for _ in range(k):
                            if allsteps:
                                allsteps.pop(0)()

                    for rt in range(8):
                        pss = [psS.tile([128, 512], f32, tag="ps",
                                        name=f"ps_{h}_{rt}_{lc}") for lc in range(2)]
                        for lc in range(2):
                            nc.tensor.matmul(
                                pss[lc][:],
                                kT8[bp:bp + 64, hp, rt * 128:(rt + 1) * 128],
                                qT8[bp:bp + 64, hp, lc * 512:(lc + 1) * 512],
                                start=True, stop=False)
                            weave(1)
                        exs = []
                        for lc in range(2):
                            nc.tensor.matmul(
                                pss[lc][:],
                                i128[:], b1f[:, rt, lc * 512:(lc + 1) * 512, sub],
                                start=False, stop=False)
                            nc.tensor.matmul(
                                pss[lc][:],
                                i128[:], b2f[:, rt, lc * 512:(lc + 1) * 512, sub],
                                start=False, stop=True)
                            # exp right behind the accumulation: frees pss
                            ex = exp_p.tile([128, 512], bf16, tag=f"ex{lc}",
                                            name=f"ex_{h}_{rt}_{lc}")
                            nc.scalar.activation(
                                ex[:], pss[lc][:], AF.Exp, bias=0.0, scale=0.125)
                            exs.append(ex)
                            weave(1)
                        # pv phase 1 (lc0 only), two rt-steps late so exp
                        # (ACT) is never on the PE critical path
                        if len(pending) == 2:
                            pending.pop(0)()
                        if rt == 0:
                            if h == 1:
                                emit_trans(2)
                            elif h == 2:
                                emit_read(2)
                            elif h == 3:
                                emit_trans(3)
                            elif h == 4:
                                emit_read(3)

                        def do_pv(rt=rt, ex=exs[0]):
                            nc.tensor.matmul(
                                pc_[:], v_sb[:, rt, h, 0:65], ex[:],
                                start=(rt == 0), stop=(rt == 7))
                        pending.append(do_pv)
                        exs1.append(exs[1])
                    for pv in pending:
                        pv()
                    # ctx [65, l] f32; pv in two phases sharing 1-bank pc_
                    ctx = ctxp.tile([65, 2, 512], f32, tag="ctx", name=f"ctx_{h}")
                    nc.vector.tensor_copy(ctx[:, 0, :], pc_[:])
                    for rt in range(8):
                        nc.tensor.matmul(
                            pc_[:], v_sb[:, rt, h, 0:65], exs1[rt][:],
                            start=(rt == 0), stop=(rt == 7))
                        weave(1)
                    nc.scalar.copy(ctx[:, 1, :], pc_[:])
                    while allsteps:
                        allsteps.pop(0)()
                    # ship ctx (unnormalized, + denominator row) straight out;
                    # the divide and [l, d] transpose happen on the HOST
                    nc.scalar.dma_start(out=outT[h], in_=ctx[:])

                # software pipeline: bias reads for pairs 0,1 launch in the
                # prologue (their band writes were woven into phase A); pair
                # hp's heads carry pair hp+2's band steps.
                emit_trans(0)
                emit_read(0)
                emit_trans(1)
                emit_read(1)
                for h in range(NHL):
                    hp = h // 2
                    if hp + 2 < NPAIR:
                        bsteps = [st for t in range(8)
                                  for st in make_band_steps(hp + 2, h % 2, t)]
                    else:
                        bsteps = []
                    emit_head(h, bsteps)

    nc.compile()
    return nc


def _get_program():
    if "nc" not in _CACHE:
        _CACHE["nc"] = _build_program()
    return _CACHE["nc"]


def _make_in_maps(hidden_states, Wq, Wk, Wv, dist_emb):
    hs = np.asarray(hidden_states, dtype=np.float32)
    Wq = np.asarray(Wq, dtype=np.float32)
    Wk = np.asarray(Wk, dtype=np.float32)
    Wv = np.asarray(Wv, dtype=np.float32)
    de = np.asarray(dist_emb, dtype=np.float32)

    import ml_dtypes
    BF = ml_dtypes.bfloat16
    det = np.zeros((HS, 2048), dtype=BF)
    rdt = np.zeros((HS, 2048), dtype=BF)
    det[:, :2047] = de.T.astype(BF)
    rdt[:, :2047] = de[::-1].T.astype(BF)

    in_maps = []
    hsb = [np.ascontiguousarray(hs[b].T).astype(BF) for b in range(B)]
    for c in range(NCORES):
        b, g = c // 2, c % 2
        w = np.concatenate(
            [Wq[g * 512:(g + 1) * 512],
             Wk[g * 512:(g + 1) * 512],
             Wv[g * 512:(g + 1) * 512]], axis=0)
        wTb = np.ascontiguousarray(w.T).astype(BF)
        in_maps.append({"hsT": hsb[b], "wT": wTb, "det": det, "rdt": rdt})
    return in_maps


def _run(in_maps, trace=False):
    from concourse.bass_utils import run_bass_kernel_spmd
    nc = _get_program()
    return run_bass_kernel_spmd(nc, in_maps, list(range(NCORES)), trace=trace)


def kernel(hidden_states, attention_mask, Wq, bq, Wk, bk, Wv, bv, dist_emb):
    # attention_mask / bq / bk / bv are all-zeros per the input spec; unused.
    in_maps = _make_in_maps(hidden_states, Wq, Wk, Wv, dist_emb)
    res = _run(in_maps, trace=False)
    out = np.empty((B, S, NH * HS), dtype=np.float32)
    for c in range(NCORES):
        b, g = c // 2, c % 2
        # device returns [head, d(65), l] unnormalized with the softmax
        # denominator in row 64; normalize + untranspose here
        oT = res.results[c]["outT"]
        o = oT[:, :64, :] / oT[:, 64:65, :]
        out[b, :, g * 512:(g + 1) * 512] = o.transpose(2, 0, 1).reshape(S, 512)
    return out


# revision 47
# speedup vs baseline: 1.0286x; 1.0286x over previous
"""BertSelfAttention (relative_key_query) Trainium2 Bass kernel.

Sharding: 8 cores = 4 batches x 2 head-groups (8 heads each); each core is
fully independent (no collectives).

Score layout is TRANSPOSED on-chip: scoresT[r, l] (r on partitions), so
probs @ V needs no transpose of probs, and the softmax denominator falls
out of an appended ones-column in the PV matmul.

Relative-position bias ("relative_key_query"):
  bias1[l,r] = q[l] . dist_emb[l-r+1023]
  bias2[l,r] = k[r] . dist_emb[l-r+1023]
computed as banded K=64 row-packed matmuls against the distance tables and
realigned via a SHEARED fp8 DRAM scratch (a DRAM-side shear is the only
mechanism on TRN2 that can express the (l-r) diagonal gather).  The TWO
HEADS of a pair are packed as the two BYTES of one bf16 container:
  qb12 row l of pair p: f8 col 2*(128+r)+h%2 = bias1_h[l,r]
  kb12 row r of pair p: f8 col 2*(128+l)+h%2 = bias2_h[l,r]
so each pair reads back with ONE bf16 xbar DMA-transpose (qd) plus ONE
plain bf16 row read (kd) -> [r-part, rt, l] tiles; fp8 halves the scratch
traffic vs a bf16 layout.  The bias add happens ON THE PE as two identity
matmuls per score chunk with a stride-2 fp8 rhs selecting the byte lane
(no GPSIMD pre-add, no unpack pass).

exp(scores/8) runs on ScalarE straight out of PSUM into bf16 probs; pv runs
bf16 with the ones-column producing the denominator.  The context ships to
DRAM UNNORMALIZED and TRANSPOSED ([head, d(65), l], row 64 = denominator):
the divide and the [l, h*64+d] unshuffle happen on the host, which removes
the whole PE-transpose + reciprocal + scale output stage from the device.

Scheduling (the HAM clock gate at K=4/8 half-rate was the dominant v1
loss): phase A (projections) weaves the band matmuls for pairs 0/1 between
projection matmuls with double-buffered PSUM so evacuation latency never
enters the PE queue; its scoped pools end in a barrier that drains the
DVE/ACT evacuation backlog before the head phase starts.  Heads 0-3 carry
pairs 2/3's band steps as weave filler; per-pair bias reads prefetch two
heads ahead; pv is deferred a few steps so exp is off the PE critical
path; v-projection evacs alternate DVE/ACT so neither backlog gates head
0.  DMA rings: sync = weights + qd shears + transposes + output, scalar =
activations + kb12, gpsimd(SWDGE) = tables + kd shears.
"""

import numpy as np

B, S, H = 4, 1024, 1024
NH, HS = 16, 64
NHL = 8            # heads per core
NPAIR = 4          # head-pairs per core
BAND = 1152        # banded width of qd'/kd per 128-row tile (1151 used + 1 pad)
RSF = 2560         # scratch row pitch in f8 bytes (= 2*BAND + 256 exactly)
TSPF = 128 * RSF   # f8 elements per 128-row block
HSPF = S * RSF     # f8 elements per head-pair
NCORES = 8

_CACHE = {}


def _build_program():
    import concourse.bass as bass
    import concourse.mybir as mybir
    import concourse.tile as tile
    from concourse import bacc

    f32 = mybir.dt.float32
    bf16 = mybir.dt.bfloat16
    f8 = mybir.dt.float8e4
    AF = mybir.ActivationFunctionType
    ALU = mybir.AluOpType

    nc = bacc.Bacc("TRN2", debug=False)

    hsT = nc.dram_tensor("hsT", [H, S], bf16, kind="ExternalInput").ap()
    wT = nc.dram_tensor("wT", [H, 3 * 512], bf16, kind="ExternalInput").ap()
    det = nc.dram_tensor("det", [HS, 2048], bf16, kind="ExternalInput").ap()
    rdt = nc.dram_tensor("rdt", [HS, 2048], bf16, kind="ExternalInput").ap()
    # transposed UNNORMALIZED output [head, d, l]; row 64 is the softmax
    # denominator; the host does the divide and the [l, h*64+d] unshuffle
    outT = nc.dram_tensor("outT", [NHL, HS + 1, S], f32, kind="ExternalOutput").ap()
    # f8 scratch; row l of pair p holds bias1 for heads (2p, 2p+1) byte-packed
    qb12 = nc.dram_tensor("qb12", [NPAIR, S, RSF], f8)
    kb12 = nc.dram_tensor("kb12", [NPAIR, S, RSF], f8)

    with tile.TileContext(nc) as tc:
        # single top-level pool scope: no mid-kernel pool closes, hence no
        # cross-engine barriers between the projection and score phases
        with tc.tile_pool(name="const", bufs=1) as constp, \
             tc.tile_pool(name="qkv", bufs=1) as qkvp, \
             tc.tile_pool(name="bandp", bufs=2) as bandp, \
             tc.tile_pool(name="psB", bufs=2, space="PSUM") as psB, \
             tc.tile_pool(name="psS", bufs=2, space="PSUM") as psS, \
             tc.tile_pool(name="psC", bufs=1, space="PSUM") as psC:
            # dist tables duplicated on partitions [0:64] and [64:128] so the
            # K=64 head-pair matmuls can row-pack (lhsT/rhs same base partition)
            det_sb = constp.tile([128, 2048], bf16)
            rdt_sb = constp.tile([128, 2048], bf16)
            i128 = constp.tile([128, 128], f8)       # fp8 identity
            onesb = constp.tile([128, 1], bf16)
            nc.gpsimd.dma_start(out=det_sb[0:64, :], in_=det[:])
            nc.gpsimd.dma_start(out=det_sb[64:128, :], in_=det[:])
            nc.gpsimd.dma_start(out=rdt_sb[0:64, :], in_=rdt[:])
            nc.gpsimd.dma_start(out=rdt_sb[64:128, :], in_=rdt[:])
            nc.gpsimd.memset(i128[:], 0.0)
            nc.gpsimd.affine_select(
                out=i128[:], in_=i128[:],
                compare_op=mybir.AluOpType.not_equal,
                fill=1.0, base=0,
                pattern=[[-1, 128]], channel_multiplier=1)
            nc.vector.memset(onesb[:], 1.0)

            # persistent per-core activations (layout [part=(h%2)*64+d, h//2, l])
            qT8 = qkvp.tile([128, 4, S], bf16)
            kT8 = qkvp.tile([128, 4, S], bf16)
            # bf16 v; col 64 = ones (softmax denominator via pv matmul)
            v_sb = qkvp.tile([128, 8, NHL, 66], bf16)
            nc.gpsimd.memset(v_sb[:], 1.0)

            CHUNKS = ((0, 512), (512, 512), (1024, 128))

            def make_band_steps(hp, which, t):
                """5 composite step-closures: per chunk, both subs' MMs into
                one single-bank psB tile + ONE byte-interleaving evac; the
                last also issues the sheared f8 DRAM write.  Evac engine
                split is phase-aware: phase A (pairs 0,1) gives ACT 3/5
                chunks; the head phase (pairs 2,3) gives ACT only 1/5 so
                exp() never queues behind band evacuation."""
                src8, tab, dst = (
                    (qT8, rdt_sb, qb12) if which == 0
                    else (kT8, det_sb, kb12))
                c0 = 896 - 128 * t
                band = bandp.tile([128, 2 * BAND], f8,
                                  tag=f"band{which}",
                                  name=f"band_{hp}_{which}_{t}")
                # view [part, sub-byte, c]: element (sub, c) -> f8 col 2c+sub
                band_i = band[:].rearrange("p (c two) -> p two c", two=2)
                act_chunks = ((t + which) % 2,)
                steps = []
                for ci, (coff, w) in enumerate(CHUNKS):
                    def step(ci=ci, coff=coff, w=w, last=(ci == 2)):
                        p = psB.tile([128, 2, 512], f32, tag="pqd")
                        for sub in range(2):
                            bp = 64 * sub
                            nc.tensor.matmul(
                                p[:, sub, 0:w],
                                src8[bp:bp + 64, hp, t * 128:(t + 1) * 128],
                                tab[bp:bp + 64, c0 + coff: c0 + coff + w],
                                start=True, stop=True)
                        dst_sl = band_i[:, :, coff:coff + w]
                        if ci in act_chunks:
                            nc.scalar.copy(dst_sl, p[:, :, 0:w])
                        else:
                            nc.vector.tensor_copy(dst_sl, p[:, :, 0:w])
                        if last:
                            shear = bass.AP(
                                tensor=dst,
                                offset=hp * HSPF + t * TSPF + 2,
                                ap=[[RSF + 2, 128], [1, 2 * BAND]])
                            if which == 0:
                                nc.sync.dma_start(out=shear, in_=band[:])
                            else:
                                # SWDGE on the otherwise-idle GPSIMD queue:
                                # keeps the ACT engine free for exp/evac
                                nc.gpsimd.dma_start(out=shear, in_=band[:])
                    steps.append(step)
                return steps

            # ---------- Phase A: QKV projections ----------
            hsT_sb = qkvp.tile([128, 8, S], bf16)
            wT_sb = qkvp.tile([128, 8, 3 * 512], bf16)
            # warm-up burst: tiny matmuls keep the PE HAM counter busy
            # while the input DMAs land, so projections start at 2.4 GHz
            for wi in range(128):
                wps = psS.tile([1, 4], f32, tag="ps", name=f"warm_{wi}")
                nc.tensor.matmul(wps[:, 0:1], onesb[:, 0:1], onesb[:],
                                 start=True, stop=True)
            phase_steps = []

            def pweave():
                if phase_steps:
                    phase_steps.pop(0)()

            hsT_r = hsT.rearrange("(a p) l -> p a l", p=128)
            wT_r = wT.rearrange("(a p) n -> p a n", p=128)
            for j in range(8):
                nc.sync.dma_start(out=wT_sb[:, j, :], in_=wT_r[:, j, :])
                nc.scalar.dma_start(out=hsT_sb[:, j, :], in_=hsT_r[:, j, :])

            def emit_proj(sel, dst8):
                # qT / kT: out[o, l] = sum_j W[o, j] hs[l, j].  ALL FOUR
                # head-pairs' band steps are emitted here: 192 total steps
                # exactly fill the 192 projection weave points, so the
                # entire bias pipeline runs inside phase A and the head
                # phase is a clean PE-paced stream.
                for ot in range(4):
                    ps2 = [psS.tile([128, 512], f32, tag="ps",
                                    name=f"pa_{sel}_{ot}_{lc}")
                           for lc in range(2)]
                    for j in range(8):
                        for lc in range(2):
                            nc.tensor.matmul(
                                ps2[lc][:],
                                wT_sb[:, j, sel * 512 + ot * 128: sel * 512 + (ot + 1) * 128],
                                hsT_sb[:, j, lc * 512:(lc + 1) * 512],
                                start=(j == 0), stop=(j == 7))
                            pweave()
                    for lc in range(2):
                        nc.vector.tensor_copy(dst8[:, ot, lc * 512:(lc + 1) * 512], ps2[lc][:])
                    if ot < 2:
                        phase_steps.extend(
                            st for t in range(8)
                            for st in make_band_steps(ot, sel, t))

            emit_proj(0, qT8)
            emit_proj(1, kT8)
            for rt in range(8):
                p = psS.tile([128, 512], f32, tag="ps", name=f"pav_{rt}")
                for j in range(8):
                    nc.tensor.matmul(
                        p[:],
                        hsT_sb[:, j, rt * 128:(rt + 1) * 128],
                        wT_sb[:, j, 1024:1536],
                        start=(j == 0), stop=(j == 7))
                    pweave()
                nc.vector.tensor_copy(
                    v_sb[:, rt, :, 0:64],
                    p[:].rearrange("p (h d) -> p h d", h=NHL))
            while phase_steps:
                phase_steps.pop(0)()

            # ---------- Phases B+C interleaved ----------
            with tc.tile_pool(name="b1p", bufs=3) as b1p, \
                 tc.tile_pool(name="b2p", bufs=3) as b2p, \
                 tc.tile_pool(name="exp", bufs=9) as exp_p, \
                 tc.tile_pool(name="ctxp", bufs=2) as ctxp, \
                 tc.tile_pool(name="outp", bufs=2) as outp:

                btiles = {}

                def emit_trans(pr):
                    # bias1 for pair pr: ONE bf16 xbar transpose-read.
                    # partition p of group g holds heads (2pr, 2pr+1) byte-
                    # packed bias1[l, r=128g+p] at bf16 col l.
                    t1 = b1p.tile([128, 8, S], bf16, tag="b1", name=f"b1_{pr}")
                    src = bass.AP(
                        tensor=qb12,
                        offset=pr * HSPF + 256,
                        ap=[[RSF, S], [1, 2048]]).bitcast(bf16)
                    nc.sync.dma_start_transpose(t1[:], src)
                    btiles[(0, pr)] = t1

                def emit_read(pr):
                    # bias2 for pair pr: ONE plain bf16 read of sheared rows
                    # (sync queue: keeps the ACT engine exp-only)
                    t2 = b2p.tile([128, 8, S], bf16, tag="b2", name=f"b2_{pr}")
                    src = bass.AP(
                        tensor=kb12,
                        offset=pr * HSPF + 256,
                        ap=[[RSF, 128], [TSPF, 8], [1, 2048]]).bitcast(bf16)
                    nc.scalar.dma_start(out=t2[:], in_=src)
                    btiles[(1, pr)] = t2

                def emit_head(h, steps):
                    hp, sub = h // 2, h % 2
                    bp = 64 * sub
                    b1t = btiles[(0, hp)]
                    b2t = btiles[(1, hp)]
                    # f8 views [part, g, l, byte]; byte lane = head parity
                    b1f = b1t[:].bitcast(f8).rearrange(
                        "p g (l two) -> p g l two", two=2)
                    b2f = b2t[:].bitcast(f8).rearrange(
                        "p g (l two) -> p g l two", two=2)
                    pc_ = psC.tile([65, 512], f32, tag="pc", name=f"pc_{h}")
                    pending = []
                    exs1 = []
                    allsteps = list(steps)

                    def weave(k=1):
                        for _ in range(k):
                            if allsteps:
                                allsteps.pop(0)()

                    for rt in range(8):
                        pss = [psS.tile([128, 512], f32, tag="ps",
                                        name=f"ps_{h}_{rt}_{lc}") for lc in range(2)]
                        for lc in range(2):
                            nc.tensor.matmul(
                                pss[lc][:],
                                kT8[bp:bp + 64, hp, rt * 128:(rt + 1) * 128],
                                qT8[bp:bp + 64, hp, lc * 512:(lc + 1) * 512],
                                start=True, stop=False)
                            weave(1)
                        exs = []
                        for lc in range(2):
                            nc.tensor.matmul(
                                pss[lc][:],
                                i128[:], b1f[:, rt, lc * 512:(lc + 1) * 512, sub],
                                start=False, stop=False)
                            nc.tensor.matmul(
                                pss[lc][:],
                                i128[:], b2f[:, rt, lc * 512:(lc + 1) * 512, sub],
                                start=False, stop=True)
                            # exp right behind the accumulation: frees pss
                            ex = exp_p.tile([128, 512], bf16, tag=f"ex{lc}",
                                            name=f"ex_{h}_{rt}_{lc}")
                            nc.scalar.activation(
                                ex[:], pss[lc][:], AF.Exp, bias=0.0, scale=0.125)
                            exs.append(ex)
                            weave(1)
                        # pv phase 1 (lc0 only), two rt-steps late so exp
                        # (ACT) is never on the PE critical path
                        if len(pending) == 2:
                            pending.pop(0)()
                        if rt == 0:
                            if h == 1:
                                emit_trans(2)
                            elif h == 2:
                                emit_read(2)
                            elif h == 3:
                                emit_trans(3)
                            elif h == 4:
                                emit_read(3)

                        def do_pv(rt=rt, ex=exs[0]):
                            nc.tensor.matmul(
                                pc_[:], v_sb[:, rt, h, 0:65], ex[:],
                                start=(rt == 0), stop=(rt == 7))
                        pending.append(do_pv)
                        exs1.append(exs[1])
                    for pv in pending:
                        pv()
                    # ctx [65, l] f32; pv in two phases sharing 1-bank pc_
                    ctx = ctxp.tile([65, 2, 512], f32, tag="ctx", name=f"ctx_{h}")
                    nc.vector.tensor_copy(ctx[:, 0, :], pc_[:])
                    for rt in range(8):
                        nc.tensor.matmul(
                            pc_[:], v_sb[:, rt, h, 0:65], exs1[rt][:],
                            start=(rt == 0), stop=(rt == 7))
                        weave(1)
                    nc.scalar.copy(ctx[:, 1, :], pc_[:])
                    while allsteps:
                        allsteps.pop(0)()
                    # ship ctx (unnormalized, + denominator row) straight out;
                    # the divide and [l, d] transpose happen on the HOST
                    nc.sync.dma_start(out=outT[h], in_=ctx[:])

                # software pipeline: bias reads for pairs 0,1 launch in the
                # prologue (their band writes were woven into phase A); pair
                # hp's heads carry pair hp+2's band steps.
                emit_trans(0)
                emit_read(0)
                emit_trans(1)
                emit_read(1)
                for h in range(NHL):
                    hp = h // 2
                    if hp + 2 < NPAIR:
                        bsteps = [st for t in range(8)
                                  for st in make_band_steps(hp + 2, h % 2, t)]
                    else:
                        bsteps = []
                    emit_head(h, bsteps)

    nc.compile()
    return nc


def _get_program():
    if "nc" not in _CACHE:
        _CACHE["nc"] = _build_program()
    return _CACHE["nc"]


def _make_in_maps(hidden_states, Wq, Wk, Wv, dist_emb):
    hs = np.asarray(hidden_states, dtype=np.float32)
    Wq = np.asarray(Wq, dtype=np.float32)
    Wk = np.asarray(Wk, dtype=np.float32)
    Wv = np.asarray(Wv, dtype=np.float32)
    de = np.asarray(dist_emb, dtype=np.float32)

    import ml_dtypes
    BF = ml_dtypes.bfloat16
    det = np.zeros((HS, 2048), dtype=BF)
    rdt = np.zeros((HS, 2048), dtype=BF)
    det[:, :2047] = de.T.astype(BF)
    rdt[:, :2047] = de[::-1].T.astype(BF)

    in_maps = []
    hsb = [np.ascontiguousarray(hs[b].T).astype(BF) for b in range(B)]
    for c in range(NCORES):
        b, g = c // 2, c % 2
        w = np.concatenate(
            [Wq[g * 512:(g + 1) * 512],
             Wk[g * 512:(g + 1) * 512],
             Wv[g * 512:(g + 1) * 512]], axis=0)
        wTb = np.ascontiguousarray(w.T).astype(BF)
        in_maps.append({"hsT": hsb[b], "wT": wTb, "det": det, "rdt": rdt})
    return in_maps


def _run(in_maps, trace=False):
    from concourse.bass_utils import run_bass_kernel_spmd
    nc = _get_program()
    return run_bass_kernel_spmd(nc, in_maps, list(range(NCORES)), trace=trace)


def kernel(hidden_states, attention_mask, Wq, bq, Wk, bk, Wv, bv, dist_emb):
    # attention_mask / bq / bk / bv are all-zeros per the input spec; unused.
    in_maps = _make_in_maps(hidden_states, Wq, Wk, Wv, dist_emb)
    res = _run(in_maps, trace=False)
    out = np.empty((B, S, NH * HS), dtype=np.float32)
    for c in range(NCORES):
        b, g = c // 2, c % 2
        # device returns [head, d(65), l] unnormalized with the softmax
        # denominator in row 64; normalize + untranspose here
        oT = res.results[c]["outT"]
        o = oT[:, :64, :] / oT[:, 64:65, :]
        out[b, :, g * 512:(g + 1) * 512] = o.transpose(2, 0, 1).reshape(S, 512)
    return out
